# revision 1
# baseline (speedup 1.0000x reference)
"""Trainium2 Bass kernel for nn_CompressionAugmentedTrainer.

Strategy (8-core SPMD, channel-sharded):
- Shard C=64 channels across 8 cores (8 ch/core, 512 rows/core); W row-sharded
  to match; partial features all-reduced (tiny [384,512]) before the loss tail.
- The three spectral views are circular convolutions along T (the magnitude
  masking in the reference is linear for real input). Each is computed as a
  circulant matmul using only 32 distinct 128x128 lhsT tiles (the circulant's
  block-diagonal structure), at fp32r full PE rate.
- noisy / combined views use linearity: f(x + s*n) = f(x) + f(s*n), with the
  per-row stds computed on-chip via ones-vector partition-sum matmuls.
- Views are packed in pairs into [128,1024] tiles so feature matmuls run with
  M=128 (two views per matmul) against each streamed W chunk (W read once).
"""
import numpy as np

B, C, T, D = 64, 64, 4096, 512
N_CORES = 8
CH = C // N_CORES            # 8 channels per core
R = B * CH                   # 512 rows per core
RB = R // 128                # 4 row blocks
TBS = T // 128               # 32 t blocks
NOISE_STD = 0.02
TEMP = 0.1
NV = 5                       # views
AR_ROWS = 6 * 64             # x, comp, dist, noisy, zt, n2f partials

_NC_CACHE = {}


def _host_consts(freq_start, time_start):
    k = np.arange(T)
    keep2048 = (k < int(T * 0.5)).astype(np.float64)
    keep3072 = (k < int(T * 0.75)).astype(np.float64)
    fw = int(0.1 * T)
    fmask = np.where((k >= freq_start) & (k < freq_start + fw), 0.1, 1.0)
    tw = int(0.05 * T)
    tmask = np.where((k >= time_start) & (k < time_start + tw), 0.1, 1.0)
    m1s = (keep3072 + keep3072[(-k) % T]) / 2.0

    cs = [np.real(np.fft.ifft(m)) for m in
          (keep2048, fmask, m1s * fmask)]

    # circulant lhsT tiles: kern[v, d, j, i] = c_v[(128*d + i - j) % T]
    dd = np.arange(TBS)[:, None, None]
    jj = np.arange(128)[None, :, None]
    ii = np.arange(128)[None, None, :]
    idx = (128 * dd + ii - jj) % T
    kern = np.stack([c[idx] for c in cs]).astype(np.float32)  # [3,32,128,128]

    # t-mask per-partition columns for every affected tb
    tb_aff = sorted({t // 128 for t in range(time_start, time_start + tw)})
    tcols = np.stack([tmask[tb * 128:(tb + 1) * 128] for tb in tb_aff],
                     axis=1).astype(np.float32)              # [128, n_aff]

    n = NV * B
    maskmat = (np.eye(n, k=1) + np.eye(n, k=-1)).astype(np.float32)
    cnt = maskmat.sum(1, keepdims=True).astype(np.float32)   # [320, 1]
    return kern, tb_aff, tcols, maskmat, cnt


def _build_nc(tb_aff, n_cores, use_collective):
    import concourse.bacc as bacc
    import concourse.mybir as mybir
    import concourse.tile as tile
    from concourse.masks import make_identity

    DT = mybir.dt.float32
    F32R = mybir.dt.float32r
    AF = mybir.ActivationFunctionType
    n_aff = len(tb_aff)

    nc = bacc.Bacc("TRN2", target_bir_lowering=False, debug=False,
                   num_devices=n_cores)

    xs_d = nc.dram_tensor("xs", [R, T], DT, kind="ExternalInput").ap()
    n1_d = nc.dram_tensor("n1s", [R, T], DT, kind="ExternalInput").ap()
    n2_d = nc.dram_tensor("n2s", [R, T], DT, kind="ExternalInput").ap()
    w_d = nc.dram_tensor("Ws", [CH * T, D], DT, kind="ExternalInput").ap()
    bias_d = nc.dram_tensor("bias", [1, D], DT, kind="ExternalInput").ap()
    kern_d = nc.dram_tensor("kern", [3 * TBS, 128, 128], DT,
                            kind="ExternalInput").ap()
    tm_d = nc.dram_tensor("tmaskc", [128, n_aff], DT, kind="ExternalInput").ap()
    mm_d = nc.dram_tensor("maskmat", [NV * B, NV * B], DT,
                          kind="ExternalInput").ap()
    cnt_d = nc.dram_tensor("cnt", [NV * B, 1], DT, kind="ExternalInput").ap()
    out_d = nc.dram_tensor("out_loss", [1, 1], DT, kind="ExternalOutput").ap()

    zt_d = nc.dram_tensor("zt_bounce", [TBS, 128, D], DT).ap()
    s1_d = nc.dram_tensor("s1_bounce", [1, R], DT).ap()
    s2_d = nc.dram_tensor("s2_bounce", [1, R], DT).ap()
    ar_in = nc.dram_tensor("ar_in", [AR_ROWS, D], DT).ap()
    ar_out = nc.dram_tensor("ar_out", [AR_ROWS, D], DT,
                            addr_space="Shared").ap()

    def mm(out, lhsT, rhs, start, stop):
        nc.tensor.matmul(out, lhsT.bitcast(F32R), rhs.bitcast(F32R),
                         start=start, stop=stop)

    with tile.TileContext(nc) as tc:
      with tc.tile_pool(name="const", bufs=1) as cp:
        kern_sb = cp.tile([128, 3 * TBS * 128], DT, tag="kern")
        nc.sync.dma_start(
            kern_sb[:].rearrange("j (g i) -> j g i", i=128).bitcast(F32R),
            kern_d.rearrange("g j i -> j g i").bitcast(F32R))
        ident = cp.tile([128, 128], DT, tag="ident")
        make_identity(nc, ident[:])
        ones_raw = cp.tile([128, 1], DT, tag="ones_raw")
        nc.vector.memset(ones_raw[:], 1.0)
        ones = cp.tile([128, 1], DT, tag="ones")
        nc.scalar.copy(ones[:].bitcast(F32R), ones_raw[:])
        tmc = cp.tile([128, n_aff], DT, tag="tmc")
        nc.sync.dma_start(tmc[:], tm_d)
        xt_sb = cp.tile([128, TBS * D], DT, tag="xt")
        s1b = cp.tile([128, R], DT, tag="s1b")
        s2b = cp.tile([128, R], DT, tag="s2b")

        def kslice(v, d):
            return kern_sb[:, (v * TBS + d) * 128:(v * TBS + d + 1) * 128]

        def xslice(tb):
            return xt_sb[:, tb * D:(tb + 1) * D]

        with tc.tile_pool(name="fps", bufs=1, space="PSUM") as fps:
            # ---------- Phase A: load x, transpose to [t, r], row stats ----------
            with (
                tc.tile_pool(name="pa_sb", bufs=1) as pa,
                tc.tile_pool(name="pa_ps", bufs=1, space="PSUM") as paps,
            ):
                sum_t = pa.tile([128, RB], DT, tag="sum")
                ss_t = pa.tile([128, RB], DT, tag="ss")
                for rb in range(RB):
                    xn = pa.tile([128, T], DT, tag="nat", bufs=2)
                    nc.sync.dma_start(xn[:], xs_d[rb * 128:(rb + 1) * 128, :])
                    nc.vector.tensor_reduce(sum_t[:, rb:rb + 1], xn[:],
                                            mybir.AxisListType.X,
                                            mybir.AluOpType.add)
                    sqs = pa.tile([128, T], DT, tag="sqs", bufs=2)
                    nc.scalar.activation(sqs[:], xn[:], AF.Square,
                                         accum_out=ss_t[:, rb:rb + 1])
                    for tb in range(TBS):
                        ptr = paps.tile([128, 128], DT, tag="tr", bufs=3)
                        nc.tensor.transpose(
                            ptr[:], xn[:, tb * 128:(tb + 1) * 128], ident[:])
                        nc.scalar.copy(
                            xt_sb[:, tb * D + rb * 128:
                                  tb * D + (rb + 1) * 128].bitcast(F32R),
                            ptr[:])
                # s1 = 0.02 * sqrt((ss - sum^2/T) / (T-1))
                v1 = pa.tile([128, RB], DT, tag="v1")
                nc.vector.tensor_mul(v1[:], sum_t[:], sum_t[:])
                nc.vector.tensor_scalar_mul(v1[:], v1[:], -1.0 / T)
                nc.vector.tensor_add(v1[:], v1[:], ss_t[:])
                nc.scalar.activation(v1[:], v1[:], AF.Sqrt, scale=1.0 / (T - 1))
                nc.scalar.mul(v1[:], v1[:], NOISE_STD)
                nc.sync.dma_start(
                    s1_d[0].rearrange("(b a) -> a b", a=128), v1[:])
                nc.gpsimd.dma_start(out=s1b[:], in_=s1_d.to_broadcast((128, R)))

            # ---------- Phase B: combined-view circulant (M3) + stats ----------
            with (
                tc.tile_pool(name="pb_sb", bufs=1) as pb,
                tc.tile_pool(name="pb_ps", bufs=1, space="PSUM") as pbps,
            ):
                zsum_ps = pbps.tile([1, R], DT, tag="zsum")
                zss_ps = pbps.tile([1, R], DT, tag="zss")
                for tb in range(TBS):
                    zp = pbps.tile([128, D], DT, tag="circ", bufs=2)
                    for kb in range(TBS):
                        d = (tb - kb) % TBS
                        mm(zp[:], kslice(2, d), xslice(kb), kb == 0, kb == TBS - 1)
                    ztt = pb.tile([128, D], DT, tag="ztt", bufs=2)
                    nc.scalar.copy(ztt[:].bitcast(F32R), zp[:])
                    if tb in tb_aff:
                        nc.vector.tensor_scalar_mul(
                            ztt[:].bitcast(F32R), ztt[:],
                            tmc[:, tb_aff.index(tb):tb_aff.index(tb) + 1])
                    zsq = pb.tile([128, D], DT, tag="zsq", bufs=2)
                    nc.vector.tensor_mul(zsq[:].bitcast(F32R), ztt[:], ztt[:])
                    mm(zsum_ps[:], ones[:], ztt[:], tb == 0, tb == TBS - 1)
                    mm(zss_ps[:], ones[:], zsq[:], tb == 0, tb == TBS - 1)
                    nc.sync.dma_start(zt_d[tb], ztt[:])
                # s2 = 0.02 * sqrt((ss - sum^2/T)/(T-1)) on [1, R] rows
                zsum = pb.tile([1, R], DT, tag="zsumsb")
                zss = pb.tile([1, R], DT, tag="zsssb")
                nc.scalar.copy(zsum[:], zsum_ps[:])
                nc.scalar.copy(zss[:], zss_ps[:])
                nc.vector.tensor_mul(zsum[:], zsum[:], zsum[:])
                nc.vector.tensor_scalar_mul(zsum[:], zsum[:], -1.0 / T)
                nc.vector.tensor_add(zsum[:], zsum[:], zss[:])
                nc.scalar.activation(zsum[:], zsum[:], AF.Sqrt,
                                     scale=1.0 / (T - 1))
                nc.scalar.mul(zsum[:], zsum[:], NOISE_STD)
                nc.sync.dma_start(s2_d, zsum[:])
                nc.gpsimd.dma_start(out=s2b[:], in_=s2_d.to_broadcast((128, R)))

            # ---------- Phase C: main loop ----------
            f1_ps = fps.tile([128, D], DT, tag="f1")   # x | noisy
            f2_ps = fps.tile([128, D], DT, tag="f2")   # comp | dist
            f3_ps = fps.tile([128, D], DT, tag="f3")   # zt | n2f
            with (
                tc.tile_pool(name="pc_sb", bufs=1) as pc,
                tc.tile_pool(name="pc_ps", bufs=1, space="PSUM") as pcps,
            ):
                for tb in range(TBS):
                    p1 = pc.tile([128, 2 * D], DT, tag="p1", bufs=2)
                    p2 = pc.tile([128, 2 * D], DT, tag="p2", bufs=2)
                    p3 = pc.tile([128, 2 * D], DT, tag="p3", bufs=2)
                    nc.scalar.copy(p1[:, 0:D].bitcast(F32R), xslice(tb))
                    nc.sync.dma_start(p3[:, 0:D].bitcast(F32R), zt_d[tb].bitcast(F32R))
                    for rb in range(RB):
                        sl = slice(D + rb * 128, D + (rb + 1) * 128)
                        nb1 = pc.tile([128, 128], DT, tag="nb1", bufs=3)
                        nc.sync.dma_start(
                            nb1[:], n1_d[rb * 128:(rb + 1) * 128,
                                         tb * 128:(tb + 1) * 128])
                        tp1 = pcps.tile([128, 128], DT, tag="trc", bufs=2)
                        nc.tensor.transpose(tp1[:], nb1[:], ident[:])
                        nc.vector.tensor_mul(p1[:, sl].bitcast(F32R), tp1[:],
                                             s1b[:, rb * 128:(rb + 1) * 128])
                        nc.vector.tensor_add(
                            p1[:, sl].bitcast(F32R), p1[:, sl],
                            xt_sb[:, tb * D + rb * 128:tb * D + (rb + 1) * 128])
                        nb2 = pc.tile([128, 128], DT, tag="nb2", bufs=3)
                        nc.sync.dma_start(
                            nb2[:], n2_d[rb * 128:(rb + 1) * 128,
                                         tb * 128:(tb + 1) * 128])
                        tp2 = pcps.tile([128, 128], DT, tag="trc", bufs=2)
                        nc.tensor.transpose(tp2[:], nb2[:], ident[:])
                        nc.vector.tensor_mul(p3[:, sl].bitcast(F32R), tp2[:],
                                             s2b[:, rb * 128:(rb + 1) * 128])
                    for v, half in ((0, 0), (1, 1)):
                        zp = pcps.tile([128, D], DT, tag="circ", bufs=2)
                        for kb in range(TBS):
                            d = (tb - kb) % TBS
                            mm(zp[:], kslice(v, d), xslice(kb),
                               kb == 0, kb == TBS - 1)
                        nc.scalar.copy(p2[:, half * D:(half + 1) * D].bitcast(F32R), zp[:])
                    if tb in tb_aff:
                        nc.vector.tensor_scalar_mul(
                            p2[:, D:2 * D].bitcast(F32R), p2[:, D:2 * D],
                            tmc[:, tb_aff.index(tb):tb_aff.index(tb) + 1])
                    for cl in range(CH):
                        wch = pc.tile([128, D], DT, tag="w", bufs=4)
                        nc.sync.dma_start(
                            wch[:].bitcast(F32R),
                            w_d[cl * T + tb * 128:
                                cl * T + (tb + 1) * 128, :].bitcast(F32R))
                        st = tb == 0 and cl == 0
                        sp = tb == TBS - 1 and cl == CH - 1
                        for pt, fp in ((p1, f1_ps), (p2, f2_ps), (p3, f3_ps)):
                            lhs = pt[:].rearrange("p (v b c) -> p v b c",
                                                  v=2, c=CH)[:, :, :, cl]
                            mm(fp[:], lhs, wch[:], st, sp)

            # ---------- Phase D: all-reduce partial features ----------
            with tc.tile_pool(name="pd_sb", bufs=1) as pd:
                fsb = [pd.tile([128, D], DT, tag=f"fsb{i}", name=f"fsb{i}")
                   for i in range(3)]
                nc.scalar.copy(fsb[0][:], f1_ps[:])
                nc.scalar.copy(fsb[1][:], f2_ps[:])
                nc.scalar.copy(fsb[2][:], f3_ps[:])
                # ar rows: 0:64 x, 64:128 comp, 128:192 dist, 192:256 noisy,
                #          256:320 zt, 320:384 n2f
                nc.gpsimd.dma_start(ar_in[0:64], fsb[0][0:64, :])
                nc.gpsimd.dma_start(ar_in[192:256], fsb[0][64:128, :])
                nc.gpsimd.dma_start(ar_in[64:128], fsb[1][0:64, :])
                nc.gpsimd.dma_start(ar_in[128:192], fsb[1][64:128, :])
                nc.gpsimd.dma_start(ar_in[256:320], fsb[2][0:64, :])
                nc.gpsimd.dma_start(ar_in[320:384], fsb[2][64:128, :])
                if use_collective:
                    nc.gpsimd.collective_compute(
                        "AllReduce", mybir.AluOpType.add,
                        replica_groups=[list(range(n_cores))],
                        ins=[ar_in], outs=[ar_out])
                else:
                    nc.gpsimd.dma_start(ar_out, ar_in)

        # ---------- Phase E: loss tail (identical on every core) ----------
        with (
            tc.tile_pool(name="pe_sb", bufs=1) as pe,
            tc.tile_pool(name="pe_ps", bufs=1, space="PSUM") as peps,
        ):
            bb = pe.tile([128, D], DT, tag="bb")
            nc.gpsimd.dma_start(out=bb[:], in_=bias_d.to_broadcast((128, D)))
            fv = [pe.tile([64, D], DT, tag=f"fv{v}", name=f"fv{v}")
                   for v in range(NV)]
            for v in range(4):
                nc.sync.dma_start(fv[v][:], ar_out[v * 64:(v + 1) * 64])
            n2f = pe.tile([64, D], DT, tag="n2f")
            nc.sync.dma_start(fv[4][:], ar_out[256:320])
            nc.sync.dma_start(n2f[:], ar_out[320:384])
            nc.vector.tensor_add(fv[4][:], fv[4][:], n2f[:])
            for v in range(NV):
                nc.vector.tensor_add(fv[v][:], fv[v][:], bb[0:64, :])

            # consistency: sum over v of ||f0 - fv||^2
            cacc = pe.tile([64, 4], DT, tag="cacc")
            for v in range(1, NV):
                dd = pe.tile([64, D], DT, tag="dd", bufs=2)
                nc.vector.tensor_sub(dd[:], fv[v][:], fv[0][:])
                dsq = pe.tile([64, D], DT, tag="dsq", bufs=2)
                nc.scalar.activation(dsq[:], dd[:], AF.Square,
                                     accum_out=cacc[:, v - 1:v])
            cps = peps.tile([1, 4], DT, tag="smallps")
            nc.tensor.matmul(cps[:], ones[0:64, :], cacc[:],
                             start=True, stop=True)
            csb = pe.tile([1, 4], DT, tag="csb")
            nc.scalar.copy(csb[:], cps[:])
            cons = pe.tile([1, 1], DT, tag="cons")
            nc.vector.tensor_reduce(cons[:], csb[:], mybir.AxisListType.X,
                                    mybir.AluOpType.add)

            # normalize rows
            for v in range(NV):
                nrm = pe.tile([64, 1], DT, tag="nrm", bufs=2)
                scr = pe.tile([64, D], DT, tag="scr", bufs=2)
                nc.scalar.activation(scr[:], fv[v][:], AF.Square,
                                     accum_out=nrm[:])
                nc.scalar.sqrt(nrm[:], nrm[:])
                rnr = pe.tile([64, 1], DT, tag="rnr", bufs=2)
                nc.vector.reciprocal(rnr[:], nrm[:])
                nc.vector.tensor_scalar_mul(fv[v][:], fv[v][:], rnr[:])

            # fnT [d-part, 320]
            fnT = [pe.tile([128, NV * B], DT, tag=f"fnT{dc}", name=f"fnT{dc}")
                   for dc in range(4)]
            for v in range(NV):
                for dc in range(4):
                    tp = peps.tile([128, 64], DT, tag="ttr", bufs=2)
                    nc.tensor.transpose(
                        tp[:], fv[v][:, dc * 128:(dc + 1) * 128],
                        ident[0:64, 0:64])
                    nc.scalar.copy(fnT[dc][:, v * 64:(v + 1) * 64], tp[:])

            # sim rows, logsumexp, masked sums
            mrow = [0, 128, 256]
            mlen = [128, 128, 64]
            parts = []
            for rk in range(3):
                n_r = mlen[rk]
                sps = peps.tile([n_r, NV * B], DT, tag="sps", bufs=2)
                for dc in range(4):
                    lhs = fnT[dc][:, mrow[rk]:mrow[rk] + n_r]
                    nc.tensor.matmul(sps[:], lhs, fnT[dc][:],
                                     start=dc == 0, stop=dc == 3)
                sim = pe.tile([n_r, NV * B], DT, tag=f"sim{rk}")
                nc.scalar.copy(sim[:], sps[:])
                mx = pe.tile([n_r, 1], DT, tag="mx", bufs=2)
                nc.vector.tensor_reduce(mx[:], sim[:], mybir.AxisListType.X,
                                        mybir.AluOpType.max)
                nm10 = pe.tile([n_r, 1], DT, tag="nm10", bufs=2)
                nc.vector.tensor_scalar_mul(nm10[:], mx[:], -10.0)
                esc = pe.tile([n_r, NV * B], DT, tag="esc", bufs=2)
                sume = pe.tile([n_r, 1], DT, tag="sume", bufs=2)
                nc.scalar.activation(esc[:], sim[:], AF.Exp,
                                     bias=nm10[:], scale=10.0,
                                     accum_out=sume[:])
                lse = pe.tile([n_r, 1], DT, tag="lse", bufs=2)
                nc.scalar.activation(lse[:], sume[:], AF.Ln)
                m10 = pe.tile([n_r, 1], DT, tag="m10", bufs=2)
                nc.vector.tensor_scalar_mul(m10[:], mx[:], 10.0)
                nc.vector.tensor_add(lse[:], lse[:], m10[:])
                # masked raw sum
                mmt = pe.tile([n_r, NV * B], DT, tag="mmt", bufs=2)
                nc.sync.dma_start(mmt[:], mm_d[mrow[rk]:mrow[rk] + n_r, :])
                nc.vector.tensor_mul(mmt[:], mmt[:], sim[:])
                mr = pe.tile([n_r, 1], DT, tag="mr", bufs=2)
                nc.vector.tensor_reduce(mr[:], mmt[:], mybir.AxisListType.X,
                                        mybir.AluOpType.add)
                nc.vector.tensor_scalar_mul(mr[:], mr[:], 10.0)
                cntt = pe.tile([n_r, 1], DT, tag="cntt", bufs=2)
                nc.sync.dma_start(cntt[:], cnt_d[mrow[rk]:mrow[rk] + n_r, :])
                nc.vector.tensor_mul(cntt[:], cntt[:], lse[:])
                nc.vector.tensor_sub(mr[:], mr[:], cntt[:])
                parts.append(mr)
            stk = pe.tile([128, 3], DT, tag="stk")
            nc.vector.memset(stk[:], 0.0)
            nc.scalar.copy(stk[:, 0:1], parts[0][:])
            nc.scalar.copy(stk[:, 1:2], parts[1][:])
            nc.scalar.copy(stk[0:64, 2:3], parts[2][:])
            mps = peps.tile([1, 3], DT, tag="smallps")
            nc.tensor.matmul(mps[:], ones[:], stk[:], start=True, stop=True)
            msb = pe.tile([1, 3], DT, tag="msb")
            nc.scalar.copy(msb[:], mps[:])
            msum = pe.tile([1, 1], DT, tag="msum")
            nc.vector.tensor_reduce(msum[:], msb[:], mybir.AxisListType.X,
                                    mybir.AluOpType.add)

            # total = cons/(4*B*D) - 0.5 * msum / 638
            nc.scalar.mul(cons[:], cons[:], 1.0 / (4 * B * D))
            nc.scalar.mul(msum[:], msum[:], -0.5 / float(2 * NV * B - 2))
            tot = pe.tile([1, 1], DT, tag="tot")
            nc.vector.tensor_add(tot[:], cons[:], msum[:])
            nc.sync.dma_start(out_d, tot[:])

    nc.compile()
    return nc


def _get_nc(tb_aff, n_cores, use_collective):
    key = (tuple(tb_aff), n_cores, use_collective)
    if key not in _NC_CACHE:
        _NC_CACHE[key] = _build_nc(list(tb_aff), n_cores, use_collective)
    return _NC_CACHE[key]


def make_in_maps(x, W, b, noise1, noise2, freq_start, time_start):
    kern, tb_aff, tcols, maskmat, cnt = _host_consts(
        int(freq_start), int(time_start))
    x = np.asarray(x, dtype=np.float32)
    W = np.asarray(W, dtype=np.float32)
    b = np.asarray(b, dtype=np.float32)
    noise1 = np.asarray(noise1, dtype=np.float32)
    noise2 = np.asarray(noise2, dtype=np.float32)
    Wr = W.reshape(C, T, D)
    in_maps = []
    for core in range(N_CORES):
        cs = core * CH
        in_maps.append({
            "xs": np.ascontiguousarray(
                x[:, cs:cs + CH, :].reshape(R, T)),
            "n1s": np.ascontiguousarray(
                noise1[:, cs:cs + CH, :].reshape(R, T)),
            "n2s": np.ascontiguousarray(
                noise2[:, cs:cs + CH, :].reshape(R, T)),
            "Ws": np.ascontiguousarray(
                Wr[cs:cs + CH].reshape(CH * T, D)),
            "bias": b.reshape(1, D),
            "kern": kern.reshape(3 * TBS, 128, 128),
            "tmaskc": tcols,
            "maskmat": maskmat,
            "cnt": cnt,
        })
    return in_maps, tb_aff


def kernel(x, W, b, noise1, noise2, freq_start, time_start):
    from concourse.bass_utils import run_bass_kernel_spmd
    in_maps, tb_aff = make_in_maps(x, W, b, noise1, noise2,
                                   freq_start, time_start)
    nc = _get_nc(tb_aff, N_CORES, True)
    res = run_bass_kernel_spmd(nc, in_maps, core_ids=list(range(N_CORES)))
    return np.float32(res.results[0]["out_loss"].reshape(())[()])



# revision 10
# speedup vs baseline: 1.2482x; 1.2482x over previous
"""Trainium2 Bass kernel for nn_CompressionAugmentedTrainer.

Strategy (8-core SPMD, channel-sharded):
- Shard C=64 channels across 8 cores (8 ch/core, 512 rows/core); W row-sharded
  to match; partial features all-reduced (tiny [384,512]) before the loss tail.
- The three spectral views are circular convolutions along T (the magnitude
  masking in the reference is linear for real input). Each is computed as a
  circulant matmul using only 32 distinct 128x128 lhsT tiles (the circulant's
  block-diagonal structure), at fp32r full PE rate.
- noisy / combined views use linearity: f(x + s*n) = f(x) + f(s*n), with the
  per-row stds computed on-chip via ones-vector partition-sum matmuls.
- Views are packed in pairs into [128,1024] tiles so feature matmuls run with
  M=128 (two views per matmul) against each streamed W chunk (W read once).
"""
import numpy as np

B, C, T, D = 64, 64, 4096, 512
N_CORES = 8
CH = C // N_CORES            # 8 channels per core
R = B * CH                   # 512 rows per core
RB = R // 128                # 4 row blocks
TBS = T // 128               # 32 t blocks
NOISE_STD = 0.02
TEMP = 0.1
NV = 5                       # views
AR_ROWS = 6 * 64             # x, comp, dist, noisy, zt, n2f partials

_NC_CACHE = {}


def _host_consts(freq_start, time_start):
    k = np.arange(T)
    keep2048 = (k < int(T * 0.5)).astype(np.float64)
    keep3072 = (k < int(T * 0.75)).astype(np.float64)
    fw = int(0.1 * T)
    fmask = np.where((k >= freq_start) & (k < freq_start + fw), 0.1, 1.0)
    tw = int(0.05 * T)
    tmask = np.where((k >= time_start) & (k < time_start + tw), 0.1, 1.0)
    m1s = (keep3072 + keep3072[(-k) % T]) / 2.0
    del keep2048  # comp view is rank-2: 0.5 x + 0.5 mean - 0.5 (-1)^n altmean

    cs = [np.real(np.fft.ifft(m)) for m in
          (fmask, m1s * fmask)]

    # circulant lhsT tiles: kern[v, d, j, i] = c_v[(128*d + i - j) % T]
    dd = np.arange(TBS)[:, None, None]
    jj = np.arange(128)[None, :, None]
    ii = np.arange(128)[None, None, :]
    idx = (128 * dd + ii - jj) % T
    kern = np.stack([c[idx] for c in cs]).astype(np.float32)  # [2,32,128,128]

    # t-mask per-partition columns for every affected tb
    tb_aff = sorted({t // 128 for t in range(time_start, time_start + tw)})
    tcols = np.stack([tmask[tb * 128:(tb + 1) * 128] for tb in tb_aff],
                     axis=1).astype(np.float32)              # [128, n_aff]

    n = NV * B
    maskmat = (np.eye(n, k=1) + np.eye(n, k=-1)).astype(np.float32)
    cnt = maskmat.sum(1, keepdims=True).astype(np.float32)   # [320, 1]
    return kern, tb_aff, tcols, maskmat, cnt


def _build_nc(tb_aff, n_cores, use_collective):
    import concourse.bacc as bacc
    import concourse.mybir as mybir
    import concourse.tile as tile
    from concourse.masks import make_identity

    DT = mybir.dt.float32
    F32R = mybir.dt.float32r
    AF = mybir.ActivationFunctionType
    n_aff = len(tb_aff)

    nc = bacc.Bacc("TRN2", target_bir_lowering=False, debug=False,
                   num_devices=n_cores)

    xs_d = nc.dram_tensor("xs", [R, T], DT, kind="ExternalInput").ap()
    n1_d = nc.dram_tensor("n1s", [R, T], DT, kind="ExternalInput").ap()
    n2_d = nc.dram_tensor("n2s", [R, T], DT, kind="ExternalInput").ap()
    w_d = nc.dram_tensor("Ws", [CH * T, D], DT, kind="ExternalInput").ap()
    bias_d = nc.dram_tensor("bias", [1, D], DT, kind="ExternalInput").ap()
    kern_d = nc.dram_tensor("kern", [2 * TBS, 128, 128], DT,
                            kind="ExternalInput").ap()
    alt_in = nc.dram_tensor("altsign", [128, 1], DT, kind="ExternalInput").ap()
    tm_d = nc.dram_tensor("tmaskc", [128, n_aff], DT, kind="ExternalInput").ap()
    mm_d = nc.dram_tensor("maskmat", [NV * B, NV * B], DT,
                          kind="ExternalInput").ap()
    cnt_d = nc.dram_tensor("cnt", [NV * B, 1], DT, kind="ExternalInput").ap()
    out_d = nc.dram_tensor("out_loss", [1, 1], DT, kind="ExternalOutput").ap()

    zt_d = nc.dram_tensor("zt_bounce", [TBS, 128, D], DT).ap()
    s1_d = nc.dram_tensor("s1_bounce", [1, R], DT).ap()
    s2_d = nc.dram_tensor("s2_bounce", [1, R], DT).ap()
    mean_d = nc.dram_tensor("mean_bounce", [1, R], DT).ap()
    alt_d = nc.dram_tensor("alt_bounce", [1, R], DT).ap()
    ar_in = nc.dram_tensor("ar_in", [AR_ROWS, D], DT).ap()
    ar_out = nc.dram_tensor("ar_out", [AR_ROWS, D], DT,
                            addr_space="Shared").ap()

    def mm(out, lhsT, rhs, start, stop):
        nc.tensor.matmul(out, lhsT.bitcast(F32R), rhs.bitcast(F32R),
                         start=start, stop=stop)

    with tile.TileContext(nc) as tc:
      with tc.tile_pool(name="const", bufs=1) as cp:
        kern_sb = cp.tile([128, 2 * TBS * 128], DT, tag="kern")
        nc.sync.dma_start(
            kern_sb[:].rearrange("j (g i) -> j g i", i=128).bitcast(F32R),
            kern_d.rearrange("g j i -> j g i").bitcast(F32R))
        ident = cp.tile([128, 128], DT, tag="ident")
        make_identity(nc, ident[:])
        ones_raw = cp.tile([128, 1], DT, tag="ones_raw")
        nc.vector.memset(ones_raw[:], 1.0)
        ones = cp.tile([128, 1], DT, tag="ones")
        nc.scalar.copy(ones[:].bitcast(F32R), ones_raw[:])
        tmc = cp.tile([128, n_aff], DT, tag="tmc")
        nc.sync.dma_start(tmc[:], tm_d)
        altsg = cp.tile([128, 1], DT, tag="altsg")
        nc.sync.dma_start(altsg[:], alt_in)
        sa_raw = cp.tile([128, 2], DT, tag="sa_raw")
        nc.vector.memset(sa_raw[:, 0:1], 0.5 / T)
        nc.scalar.mul(sa_raw[:, 1:2], altsg[:], -0.5 / T)
        sa = cp.tile([128, 2], DT, tag="sa")
        nc.scalar.copy(sa[:].bitcast(F32R), sa_raw[:])
        xt_sb = cp.tile([128, TBS * D], DT, tag="xt")
        s1b = cp.tile([128, R], DT, tag="s1b")
        s2b = cp.tile([128, R], DT, tag="s2b")
        base = cp.tile([128, R], DT, tag="base")

        def kslice(v, d):
            return kern_sb[:, (v * TBS + d) * 128:(v * TBS + d + 1) * 128]

        def xslice(tb):
            return xt_sb[:, tb * D:(tb + 1) * D]

        with tc.tile_pool(name="fps", bufs=1, space="PSUM") as fps:
            # ---------- Phase A: load x, transpose to [t, r], row stats ----------
            with (
                tc.tile_pool(name="pa_sb", bufs=1) as pa,
                tc.tile_pool(name="pa_ps", bufs=1, space="PSUM") as paps,
            ):
                sum_t = pa.tile([128, RB], DT, tag="sum")
                ss_t = pa.tile([128, RB], DT, tag="ss")
                for rb in range(RB):
                    xn = pa.tile([128, T], DT, tag="nat", bufs=2)
                    nc.sync.dma_start(xn[:], xs_d[rb * 128:(rb + 1) * 128, :])
                    nc.vector.tensor_reduce(sum_t[:, rb:rb + 1], xn[:],
                                            mybir.AxisListType.X,
                                            mybir.AluOpType.add)
                    sqs = pa.tile([128, T], DT, tag="sqs", bufs=2)
                    nc.scalar.activation(sqs[:], xn[:], AF.Square,
                                         accum_out=ss_t[:, rb:rb + 1])
                    for tb in range(TBS):
                        ptr = paps.tile([128, 128], DT, tag="tr", bufs=3)
                        nc.tensor.transpose(
                            ptr[:], xn[:, tb * 128:(tb + 1) * 128], ident[:])
                        nc.scalar.copy(
                            xt_sb[:, tb * D + rb * 128:
                                  tb * D + (rb + 1) * 128].bitcast(F32R),
                            ptr[:])
                # s1 = 0.02 * sqrt((ss - sum^2/T) / (T-1))
                v1 = pa.tile([128, RB], DT, tag="v1")
                nc.vector.tensor_mul(v1[:], sum_t[:], sum_t[:])
                nc.vector.tensor_scalar_mul(v1[:], v1[:], -1.0 / T)
                nc.vector.tensor_add(v1[:], v1[:], ss_t[:])
                nc.scalar.activation(v1[:], v1[:], AF.Sqrt, scale=1.0 / (T - 1))
                nc.scalar.mul(v1[:], v1[:], NOISE_STD)
                nc.sync.dma_start(
                    s1_d[0].rearrange("(b a) -> a b", a=128), v1[:])
                nc.gpsimd.dma_start(out=s1b[:], in_=s1_d.to_broadcast((128, R)))
                # rank-2 comp view: row0 = 0.5*mean, row1 = -0.5*altmean
                ma_ps = paps.tile([2, R], DT, tag="ma")
                for tb in range(TBS):
                    mm(ma_ps[:], sa[:], xslice(tb), tb == 0, tb == TBS - 1)
                ma_sb = pa.tile([2, R], DT, tag="ma_sb")
                nc.scalar.copy(ma_sb[:], ma_ps[:])
                nc.sync.dma_start(mean_d, ma_sb[0:1, :])
                nc.sync.dma_start(alt_d, ma_sb[1:2, :])
                mb = pa.tile([128, R], DT, tag="mb")
                nc.gpsimd.dma_start(out=mb[:], in_=mean_d.to_broadcast((128, R)))
                ab = pa.tile([128, R], DT, tag="ab")
                nc.gpsimd.dma_start(out=ab[:], in_=alt_d.to_broadcast((128, R)))
                # base[t, r] = 0.5*mean[r] - 0.5*(-1)^t*altmean[r]
                nc.vector.tensor_scalar_mul(base[:], ab[:], altsg[:])
                nc.vector.tensor_add(base[:], base[:], mb[:])

            # ---------- Phase B: combined-view circulant (M3) + stats ----------
            with (
                tc.tile_pool(name="pb_sb", bufs=1) as pb,
                tc.tile_pool(name="pb_ps", bufs=1, space="PSUM") as pbps,
            ):
                zsum_ps = pbps.tile([1, R], DT, tag="zsum")
                zss_ps = pbps.tile([1, R], DT, tag="zss")
                for tb in range(TBS):
                    zp = pbps.tile([128, D], DT, tag="circ", bufs=2)
                    for kb in range(TBS):
                        d = (tb - kb) % TBS
                        mm(zp[:], kslice(1, d), xslice(kb), kb == 0, kb == TBS - 1)
                    ztt = pb.tile([128, D], DT, tag="ztt", bufs=2)
                    nc.scalar.copy(ztt[:].bitcast(F32R), zp[:])
                    if tb in tb_aff:
                        nc.vector.tensor_scalar_mul(
                            ztt[:].bitcast(F32R), ztt[:],
                            tmc[:, tb_aff.index(tb):tb_aff.index(tb) + 1])
                    zsq = pb.tile([128, D], DT, tag="zsq", bufs=2)
                    nc.vector.tensor_mul(zsq[:].bitcast(F32R), ztt[:], ztt[:])
                    mm(zsum_ps[:], ones[:], ztt[:], tb == 0, tb == TBS - 1)
                    mm(zss_ps[:], ones[:], zsq[:], tb == 0, tb == TBS - 1)
                    nc.sync.dma_start(zt_d[tb], ztt[:])
                # s2 = 0.02 * sqrt((ss - sum^2/T)/(T-1)) on [1, R] rows
                zsum = pb.tile([1, R], DT, tag="zsumsb")
                zss = pb.tile([1, R], DT, tag="zsssb")
                nc.scalar.copy(zsum[:], zsum_ps[:])
                nc.scalar.copy(zss[:], zss_ps[:])
                nc.vector.tensor_mul(zsum[:], zsum[:], zsum[:])
                nc.vector.tensor_scalar_mul(zsum[:], zsum[:], -1.0 / T)
                nc.vector.tensor_add(zsum[:], zsum[:], zss[:])
                nc.scalar.activation(zsum[:], zsum[:], AF.Sqrt,
                                     scale=1.0 / (T - 1))
                nc.scalar.mul(zsum[:], zsum[:], NOISE_STD)
                nc.sync.dma_start(s2_d, zsum[:])
                nc.gpsimd.dma_start(out=s2b[:], in_=s2_d.to_broadcast((128, R)))

            # ---------- Phase C: main loop ----------
            f1_ps = fps.tile([128, D], DT, tag="f1")   # x | noisy
            f2_ps = fps.tile([128, D], DT, tag="f2")   # comp | dist
            f3_ps = fps.tile([128, D], DT, tag="f3")   # zt | n2f
            with (
                tc.tile_pool(name="pc_sb", bufs=1) as pc,
                tc.tile_pool(name="pc_ps", bufs=1, space="PSUM") as pcps,
            ):
                for tb in range(TBS):
                    p1 = pc.tile([128, 2 * D], DT, tag="p1", bufs=2)
                    p2 = pc.tile([128, 2 * D], DT, tag="p2", bufs=2)
                    p3 = pc.tile([128, 2 * D], DT, tag="p3", bufs=2)
                    nc.scalar.copy(p1[:, 0:D].bitcast(F32R), xslice(tb))
                    nc.sync.dma_start(p3[:, 0:D].bitcast(F32R), zt_d[tb].bitcast(F32R))
                    for rb in range(RB):
                        sl = slice(D + rb * 128, D + (rb + 1) * 128)
                        nb1 = pc.tile([128, 128], DT, tag="nb1", bufs=3)
                        nc.sync.dma_start(
                            nb1[:], n1_d[rb * 128:(rb + 1) * 128,
                                         tb * 128:(tb + 1) * 128])
                        tp1 = pcps.tile([128, 128], DT, tag="trc", bufs=2)
                        nc.tensor.transpose(tp1[:], nb1[:], ident[:])
                        nc.vector.tensor_mul(p1[:, sl].bitcast(F32R), tp1[:],
                                             s1b[:, rb * 128:(rb + 1) * 128])
                        nc.vector.tensor_add(
                            p1[:, sl].bitcast(F32R), p1[:, sl],
                            xt_sb[:, tb * D + rb * 128:tb * D + (rb + 1) * 128])
                        nb2 = pc.tile([128, 128], DT, tag="nb2", bufs=3)
                        nc.sync.dma_start(
                            nb2[:], n2_d[rb * 128:(rb + 1) * 128,
                                         tb * 128:(tb + 1) * 128])
                        tp2 = pcps.tile([128, 128], DT, tag="trc", bufs=2)
                        nc.tensor.transpose(tp2[:], nb2[:], ident[:])
                        nc.vector.tensor_mul(p3[:, sl].bitcast(F32R), tp2[:],
                                             s2b[:, rb * 128:(rb + 1) * 128])
                    # comp half: rank-2 formula 0.5*x + base
                    nc.vector.tensor_scalar_mul(
                        p2[:, 0:D].bitcast(F32R), xslice(tb), 0.5)
                    nc.vector.tensor_add(
                        p2[:, 0:D].bitcast(F32R), p2[:, 0:D], base[:])
                    # dist half: fmask circulant (view 0)
                    zp = pcps.tile([128, D], DT, tag="circ", bufs=2)
                    for kb in range(TBS):
                        d = (tb - kb) % TBS
                        mm(zp[:], kslice(0, d), xslice(kb),
                           kb == 0, kb == TBS - 1)
                    nc.scalar.copy(p2[:, D:2 * D].bitcast(F32R), zp[:])
                    if tb in tb_aff:
                        nc.vector.tensor_scalar_mul(
                            p2[:, D:2 * D].bitcast(F32R), p2[:, D:2 * D],
                            tmc[:, tb_aff.index(tb):tb_aff.index(tb) + 1])
                    for cl in range(CH):
                        wch = pc.tile([128, D], DT, tag="w", bufs=4)
                        nc.sync.dma_start(
                            wch[:].bitcast(F32R),
                            w_d[cl * T + tb * 128:
                                cl * T + (tb + 1) * 128, :].bitcast(F32R))
                        st = tb == 0 and cl == 0
                        sp = tb == TBS - 1 and cl == CH - 1
                        for pt, fp in ((p1, f1_ps), (p2, f2_ps), (p3, f3_ps)):
                            lhs = pt[:].rearrange("p (v b c) -> p v b c",
                                                  v=2, c=CH)[:, :, :, cl]
                            mm(fp[:], lhs, wch[:], st, sp)

            # ---------- Phase D: all-reduce partial features ----------
            with tc.tile_pool(name="pd_sb", bufs=1) as pd:
                fsb = [pd.tile([128, D], DT, tag=f"fsb{i}", name=f"fsb{i}")
                   for i in range(3)]
                nc.scalar.copy(fsb[0][:], f1_ps[:])
                nc.scalar.copy(fsb[1][:], f2_ps[:])
                nc.scalar.copy(fsb[2][:], f3_ps[:])
                # ar rows: 0:64 x, 64:128 comp, 128:192 dist, 192:256 noisy,
                #          256:320 zt, 320:384 n2f
                nc.gpsimd.dma_start(ar_in[0:64], fsb[0][0:64, :])
                nc.gpsimd.dma_start(ar_in[192:256], fsb[0][64:128, :])
                nc.gpsimd.dma_start(ar_in[64:128], fsb[1][0:64, :])
                nc.gpsimd.dma_start(ar_in[128:192], fsb[1][64:128, :])
                nc.gpsimd.dma_start(ar_in[256:320], fsb[2][0:64, :])
                nc.gpsimd.dma_start(ar_in[320:384], fsb[2][64:128, :])
                if use_collective:
                    nc.gpsimd.collective_compute(
                        "AllReduce", mybir.AluOpType.add,
                        replica_groups=[list(range(n_cores))],
                        ins=[ar_in], outs=[ar_out])
                else:
                    nc.gpsimd.dma_start(ar_out, ar_in)

        # ---------- Phase E: loss tail (identical on every core) ----------
        with (
            tc.tile_pool(name="pe_sb", bufs=1) as pe,
            tc.tile_pool(name="pe_ps", bufs=1, space="PSUM") as peps,
        ):
            bb = pe.tile([128, D], DT, tag="bb")
            nc.gpsimd.dma_start(out=bb[:], in_=bias_d.to_broadcast((128, D)))
            fv = [pe.tile([64, D], DT, tag=f"fv{v}", name=f"fv{v}")
                   for v in range(NV)]
            for v in range(4):
                nc.sync.dma_start(fv[v][:], ar_out[v * 64:(v + 1) * 64])
            n2f = pe.tile([64, D], DT, tag="n2f")
            nc.sync.dma_start(fv[4][:], ar_out[256:320])
            nc.sync.dma_start(n2f[:], ar_out[320:384])
            nc.vector.tensor_add(fv[4][:], fv[4][:], n2f[:])
            for v in range(NV):
                nc.vector.tensor_add(fv[v][:], fv[v][:], bb[0:64, :])

            # consistency: sum over v of ||f0 - fv||^2
            cacc = pe.tile([64, 4], DT, tag="cacc")
            for v in range(1, NV):
                dd = pe.tile([64, D], DT, tag="dd", bufs=2)
                nc.vector.tensor_sub(dd[:], fv[v][:], fv[0][:])
                dsq = pe.tile([64, D], DT, tag="dsq", bufs=2)
                nc.scalar.activation(dsq[:], dd[:], AF.Square,
                                     accum_out=cacc[:, v - 1:v])
            cps = peps.tile([1, 4], DT, tag="smallps")
            nc.tensor.matmul(cps[:], ones[0:64, :], cacc[:],
                             start=True, stop=True)
            csb = pe.tile([1, 4], DT, tag="csb")
            nc.scalar.copy(csb[:], cps[:])
            cons = pe.tile([1, 1], DT, tag="cons")
            nc.vector.tensor_reduce(cons[:], csb[:], mybir.AxisListType.X,
                                    mybir.AluOpType.add)

            # normalize rows
            for v in range(NV):
                nrm = pe.tile([64, 1], DT, tag="nrm", bufs=2)
                scr = pe.tile([64, D], DT, tag="scr", bufs=2)
                nc.scalar.activation(scr[:], fv[v][:], AF.Square,
                                     accum_out=nrm[:])
                nc.scalar.sqrt(nrm[:], nrm[:])
                rnr = pe.tile([64, 1], DT, tag="rnr", bufs=2)
                nc.vector.reciprocal(rnr[:], nrm[:])
                nc.vector.tensor_scalar_mul(fv[v][:], fv[v][:], rnr[:])

            # fnT [d-part, 320]
            fnT = [pe.tile([128, NV * B], DT, tag=f"fnT{dc}", name=f"fnT{dc}")
                   for dc in range(4)]
            for v in range(NV):
                for dc in range(4):
                    tp = peps.tile([128, 64], DT, tag="ttr", bufs=2)
                    nc.tensor.transpose(
                        tp[:], fv[v][:, dc * 128:(dc + 1) * 128],
                        ident[0:64, 0:64])
                    nc.scalar.copy(fnT[dc][:, v * 64:(v + 1) * 64], tp[:])

            # sim rows, logsumexp, masked sums
            mrow = [0, 128, 256]
            mlen = [128, 128, 64]
            parts = []
            for rk in range(3):
                n_r = mlen[rk]
                sps = peps.tile([n_r, NV * B], DT, tag="sps", bufs=2)
                for dc in range(4):
                    lhs = fnT[dc][:, mrow[rk]:mrow[rk] + n_r]
                    nc.tensor.matmul(sps[:], lhs, fnT[dc][:],
                                     start=dc == 0, stop=dc == 3)
                sim = pe.tile([n_r, NV * B], DT, tag=f"sim{rk}")
                nc.scalar.copy(sim[:], sps[:])
                mx = pe.tile([n_r, 1], DT, tag="mx", bufs=2)
                nc.vector.tensor_reduce(mx[:], sim[:], mybir.AxisListType.X,
                                        mybir.AluOpType.max)
                nm10 = pe.tile([n_r, 1], DT, tag="nm10", bufs=2)
                nc.vector.tensor_scalar_mul(nm10[:], mx[:], -10.0)
                esc = pe.tile([n_r, NV * B], DT, tag="esc", bufs=2)
                sume = pe.tile([n_r, 1], DT, tag="sume", bufs=2)
                nc.scalar.activation(esc[:], sim[:], AF.Exp,
                                     bias=nm10[:], scale=10.0,
                                     accum_out=sume[:])
                lse = pe.tile([n_r, 1], DT, tag="lse", bufs=2)
                nc.scalar.activation(lse[:], sume[:], AF.Ln)
                m10 = pe.tile([n_r, 1], DT, tag="m10", bufs=2)
                nc.vector.tensor_scalar_mul(m10[:], mx[:], 10.0)
                nc.vector.tensor_add(lse[:], lse[:], m10[:])
                # masked raw sum
                mmt = pe.tile([n_r, NV * B], DT, tag="mmt", bufs=2)
                nc.sync.dma_start(mmt[:], mm_d[mrow[rk]:mrow[rk] + n_r, :])
                nc.vector.tensor_mul(mmt[:], mmt[:], sim[:])
                mr = pe.tile([n_r, 1], DT, tag="mr", bufs=2)
                nc.vector.tensor_reduce(mr[:], mmt[:], mybir.AxisListType.X,
                                        mybir.AluOpType.add)
                nc.vector.tensor_scalar_mul(mr[:], mr[:], 10.0)
                cntt = pe.tile([n_r, 1], DT, tag="cntt", bufs=2)
                nc.sync.dma_start(cntt[:], cnt_d[mrow[rk]:mrow[rk] + n_r, :])
                nc.vector.tensor_mul(cntt[:], cntt[:], lse[:])
                nc.vector.tensor_sub(mr[:], mr[:], cntt[:])
                parts.append(mr)
            stk = pe.tile([128, 3], DT, tag="stk")
            nc.vector.memset(stk[:], 0.0)
            nc.scalar.copy(stk[:, 0:1], parts[0][:])
            nc.scalar.copy(stk[:, 1:2], parts[1][:])
            nc.scalar.copy(stk[0:64, 2:3], parts[2][:])
            mps = peps.tile([1, 3], DT, tag="smallps")
            nc.tensor.matmul(mps[:], ones[:], stk[:], start=True, stop=True)
            msb = pe.tile([1, 3], DT, tag="msb")
            nc.scalar.copy(msb[:], mps[:])
            msum = pe.tile([1, 1], DT, tag="msum")
            nc.vector.tensor_reduce(msum[:], msb[:], mybir.AxisListType.X,
                                    mybir.AluOpType.add)

            # total = cons/(4*B*D) - 0.5 * msum / 638
            nc.scalar.mul(cons[:], cons[:], 1.0 / (4 * B * D))
            nc.scalar.mul(msum[:], msum[:], -0.5 / float(2 * NV * B - 2))
            tot = pe.tile([1, 1], DT, tag="tot")
            nc.vector.tensor_add(tot[:], cons[:], msum[:])
            nc.sync.dma_start(out_d, tot[:])

    nc.compile()
    return nc


def _get_nc(tb_aff, n_cores, use_collective):
    key = (tuple(tb_aff), n_cores, use_collective)
    if key not in _NC_CACHE:
        _NC_CACHE[key] = _build_nc(list(tb_aff), n_cores, use_collective)
    return _NC_CACHE[key]


def make_in_maps(x, W, b, noise1, noise2, freq_start, time_start):
    kern, tb_aff, tcols, maskmat, cnt = _host_consts(
        int(freq_start), int(time_start))
    x = np.asarray(x, dtype=np.float32)
    W = np.asarray(W, dtype=np.float32)
    b = np.asarray(b, dtype=np.float32)
    noise1 = np.asarray(noise1, dtype=np.float32)
    noise2 = np.asarray(noise2, dtype=np.float32)
    Wr = W.reshape(C, T, D)
    in_maps = []
    for core in range(N_CORES):
        cs = core * CH
        in_maps.append({
            "xs": np.ascontiguousarray(
                x[:, cs:cs + CH, :].reshape(R, T)),
            "n1s": np.ascontiguousarray(
                noise1[:, cs:cs + CH, :].reshape(R, T)),
            "n2s": np.ascontiguousarray(
                noise2[:, cs:cs + CH, :].reshape(R, T)),
            "Ws": np.ascontiguousarray(
                Wr[cs:cs + CH].reshape(CH * T, D)),
            "bias": b.reshape(1, D),
            "kern": kern.reshape(2 * TBS, 128, 128),
            "altsign": ((-1.0) ** np.arange(128)).reshape(128, 1)
                       .astype(np.float32),
            "tmaskc": tcols,
            "maskmat": maskmat,
            "cnt": cnt,
        })
    return in_maps, tb_aff


def kernel(x, W, b, noise1, noise2, freq_start, time_start):
    from concourse.bass_utils import run_bass_kernel_spmd
    in_maps, tb_aff = make_in_maps(x, W, b, noise1, noise2,
                                   freq_start, time_start)
    nc = _get_nc(tb_aff, N_CORES, True)
    res = run_bass_kernel_spmd(nc, in_maps, core_ids=list(range(N_CORES)))
    return np.float32(res.results[0]["out_loss"].reshape(())[()])

